# revision 1
# baseline (speedup 1.0000x reference)
"""MiniRocketFeaturesPlus Trainium2 kernel (v2).

Strategy (data-parallel over batch, 4 samples per core on 8 cores):
  - Tap-shifted slab: xm[(t*9+c), b*2048 + l] = x_pad[c, b, PAD - 4d + d*t + l]
    for t = 0..12 (117 contraction rows).  With p = 4d, the cropped half's
    window [p, p+V) is reachable from the SAME rhs columns via tap index
    t+4, so one matmul per (dilation, sample) emits all 84 kernel rows:
    cols 0..41 = h0 (full window), 42..83 = h1 (valid cols [0, V)).
  - 64/20 chunking into 3 PSUM tiles per dilation (4 samples):
      tile A = [s0 rows 0:64 @p0, s1 rows 0:64 @p64]   width 2048
      tile B = [s2 rows 0:64 @p0, s3 rows 0:64 @p64]   width 2048
      tile C = [s0 rows 64:84 @p0, s1 @p32, s2 @p64, s3 @p96] width V
    A/B h1-rows' tail cols [V, 2048) are memset to 0 in PSUM (the tap-shift
    makes them read real data, not padding).
  - PPV reductions straight off PSUM: cnt passes (DVE is_gt / ACT Sign) and
    rel passes (ACT Relu / DVE max), one per (tile, feature), statically
    load-balanced across both engines.  The plain sums S are computed on the
    host from prefix sums of the bf16-cast input (exact to fp32 rounding).
  - Decode on host: a = N/L, b = R / max(2R - (S - L t), 1e-8).
"""

import numpy as np
import sys

sys.path.insert(0, "/opt/trn_rl_repo")

C_IN, SEQ_LEN = 9, 2048
KERNEL_SIZE = 9
NUM_KERNELS = 84
B = 32
N_CORES = 8
B_CORE = B // N_CORES
PAD = 1020
NTAP = 13
KDIM = NTAP * C_IN  # 117
LP2 = PAD + 8 * 255 + SEQ_LEN + 4  # 5111: max read PAD+8d+2047
XCOLS = B_CORE * SEQ_LEN  # 8192
FP32_DILS = (24, 25)

# measured per-pass costs (ns) for a width-w scan
DVE_NSCOL = 1.0417
ACT_NSCOL = 0.8333
DVE_OVH = 230.0
ACT_OVH = 420.0  # activate overhead + read_accumulator


def _config():
    nf_total = 10000 // 2 // NUM_KERNELS * NUM_KERNELS
    nfpk = nf_total // NUM_KERNELS
    true_max = min(nfpk, 32)
    multiplier = nfpk / true_max
    max_exp = np.log2((SEQ_LEN - 1) / (KERNEL_SIZE - 1))
    dilations, counts = np.unique(
        np.logspace(0, max_exp, true_max, base=2).astype(np.int32),
        return_counts=True)
    nfpd = (counts * multiplier).astype(np.int32)
    rem = nfpk - nfpd.sum()
    i = 0
    while rem > 0:
        nfpd[i] += 1
        rem -= 1
        i = (i + 1) % len(nfpd)
    paddings = [(KERNEL_SIZE - 1) * int(d) // 2 for d in dilations]
    return [int(d) for d in dilations], paddings, [int(n) for n in nfpd]


DILS, PADS, NFPD = _config()
ND = len(DILS)
# process order: interleave the high-nf dilations (long single-engine
# tile monopolies) with cheap nf=1 dilations, and put the double-DMA
# fp32 dilations next to big compute phases that hide their transfers
DIL_ORDER = [0, 24, 21, 1, 25, 22, 4, 23] + [i for i in range(ND)
                                             if i not in (0, 1, 4, 21, 22,
                                                          23, 24, 25)]


def _halves(i):
    p1 = i % 2
    h0 = list(range(p1, NUM_KERNELS, 2))
    h1 = list(range(1 - p1, NUM_KERNELS, 2))
    return h0, h1


class Plan2:
    """Static schedule: tiles, passes, engine assignment, decode maps."""

    def __init__(self):
        # kernel-slot layout per dil: cols 0..41 h0, 42..83 h1
        # tiles: A (s0,s1 rows0:64), B (s2,s3 rows0:64), C (rows64:84 x4)
        # passes: per (dil, tile, f): cnt + rel
        self.passes = []  # dicts
        eng_t = {"dve": 0.0, "act": 0.0}
        ncol = {"dve": 0, "act": 0}
        nthr = 0
        self.tile_eng = {}
        for i in DIL_ORDER:
            nf = NFPD[i]
            V = SEQ_LEN - 2 * PADS[i]
            for tile in ("A", "B", "C"):
                w = SEQ_LEN if tile in ("A", "B") else V
                # whole tile goes to one engine: PSUM-tile readers serialize
                cd = 2 * nf * (w * DVE_NSCOL + DVE_OVH)
                ca = 2 * nf * (w * ACT_NSCOL + ACT_OVH)
                if eng_t["dve"] + cd <= eng_t["act"] + ca:
                    eng = "dve"
                    eng_t["dve"] += cd
                else:
                    eng = "act"
                    eng_t["act"] += ca
                self.tile_eng[(i, tile)] = eng
                for f in range(nf):
                    for typ in ("cnt", "rel"):
                        pa = dict(dil=i, tile=tile, f=f, typ=typ, eng=eng,
                                  thr=nthr, acc=ncol[eng], w=w)
                        nthr += 1
                        ncol[eng] += 1
                        self.passes.append(pa)
        self.nthr = nthr
        self.ncol_dve = ncol["dve"]
        self.ncol_act = ncol["act"]
        self.est_dve = eng_t["dve"]
        self.est_act = eng_t["act"]
        self.off = np.concatenate([[0], np.cumsum(NFPD)]).astype(int)

    # --- partition layout -------------------------------------------------
    # tile A/B partition p: sample = sA + (p >= 64); r = p % 64
    #   r in [0,22): h1[r]; r in [22,64): h0[r-22]
    # tile C partition p: sample = p // 32; r = p % 32 (valid r < 20): h1[22+r]
    def thr_col(self, i, f, typ, eng, tile, biases):
        h0, h1 = _halves(i)
        t_of = np.zeros(128, np.float32)
        col = np.zeros(128, np.float32)
        use = np.zeros(128, bool)
        if tile in ("A", "B"):
            for half in range(2):
                for r in range(22):
                    t_of[half * 64 + r] = biases[h1[r], self.off[i] + f]
                    use[half * 64 + r] = True
                for r in range(42):
                    t_of[half * 64 + 22 + r] = biases[h0[r], self.off[i] + f]
                    use[half * 64 + 22 + r] = True
        else:
            for s in range(4):
                for r in range(20):
                    t_of[s * 32 + r] = biases[h1[22 + r], self.off[i] + f]
                    use[s * 32 + r] = True
        t = t_of.astype(np.float32)
        if typ == "cnt":
            if eng == "dve":
                col = t.copy()  # is_gt(c, t)
            else:
                tp = np.nextafter(t, np.float32(np.inf)).astype(np.float32)
                col = -tp  # Sign(c - t')
        else:
            if eng == "dve":
                col = t.copy()  # max(c, t)
            else:
                col = -t  # Relu(c - t)
        col[~use] = 0.0
        return col

    def build_thresholds(self, biases):
        biases = np.asarray(biases, np.float32)
        thr = np.zeros((128, self.nthr), np.float32)
        for pa in self.passes:
            thr[:, pa["thr"]] = self.thr_col(pa["dil"], pa["f"], pa["typ"],
                                             pa["eng"], pa["tile"], biases)
        return thr

    # --- host-side S (exact window sums of bf16-cast data) ----------------
    def host_S(self, x_pad_bf, w2t):
        """S[i][half, j, core, b]: window sums matching device C.
        x_pad_bf: [B, C_IN, LP2] fp32 (bf16-cast values), w2t: [ND, 117, 84]
        (bf16-cast values, fp64).  Returns dict i -> [2, 42, ncore, B_CORE]."""
        P = np.zeros((B, C_IN, LP2 + 1), np.float64)
        np.cumsum(x_pad_bf.astype(np.float64), axis=2, out=P[:, :, 1:])
        S = {}
        for i, (d, p, nf) in enumerate(zip(DILS, PADS, NFPD)):
            V = SEQ_LEN - 2 * p
            # xsum[k=(t,c), b_global] over window h0: [o, o+2048), o=PAD-4d+dt
            o0 = np.array([PAD - 4 * d + d * t for t in range(NTAP)])
            # h0 uses taps 0..8 rows t*9+c; h1 uses rows (t+4)*9+c win [0,V)
            s_arr = np.zeros((2, 42, B), np.float64)
            w = w2t[i]  # [117, 84] fp64 (bf16 values)
            # windows sums for all 117 rows, both window types
            xs0 = np.zeros((KDIM, B), np.float64)  # h0 window [0, 2048)
            xs1 = np.zeros((KDIM, B), np.float64)  # h1 window [0, V)
            for t in range(NTAP):
                o = o0[t]
                for c in range(C_IN):
                    xs0[t * 9 + c] = P[:, c, o + SEQ_LEN] - P[:, c, o]
                    xs1[t * 9 + c] = P[:, c, o + V] - P[:, c, o]
            s_arr[0] = w[:, 22:64].T @ xs0  # h0 slots
            s_arr[1, 0:22] = w[:, 0:22].T @ xs1  # h1[0:22]
            s_arr[1, 22:42] = w[:, 64:84].T @ xs1  # h1[22:42]
            S[i] = s_arr
        return S

    # --- decode -----------------------------------------------------------
    def decode(self, acc_dve, acc_act, biases, S):
        """acc_*: [ncores, 128, ncol_*] -> [B, 9912] fp32."""
        biases = np.asarray(biases, np.float64)
        nfs = NFPD
        blk_off = np.concatenate([[0], np.cumsum([168 * nf for nf in nfs])]).astype(int)
        out = np.zeros((B, blk_off[-1]), np.float64)
        # gather N, R per (dil, f, half-slot, j-slot, core, sample)
        N = {}
        R = {}
        for pa in self.passes:
            i, f, tile, typ, eng = pa["dil"], pa["f"], pa["tile"], pa["typ"], pa["eng"]
            V = SEQ_LEN - 2 * PADS[i]
            acc = (acc_dve if eng == "dve" else acc_act)[:, :, pa["acc"]]  # [ncores,128]
            key = (i, f)
            if key not in N:
                N[key] = np.full((2, 42, N_CORES, B_CORE), np.nan)
                R[key] = np.full((2, 42, N_CORES, B_CORE), np.nan)
            tgt = N[key] if typ == "cnt" else R[key]
            h0, h1 = _halves(i)
            if tile in ("A", "B"):
                sbase = 0 if tile == "A" else 2
                for half in range(2):
                    s = sbase + half
                    v = acc[:, half * 64: half * 64 + 64]  # [ncores, 64]
                    # h1 rows r 0..21 -> h1[0:22]: valid V + zeros tail
                    raw1 = v[:, 0:22]
                    # h0 rows r 22..63: scan 2048, fully valid
                    raw0 = v[:, 22:64]
                    t0 = biases[h0, self.off[i] + f][None, :]
                    t1 = biases[h1[0:22], self.off[i] + f][None, :]
                    T2 = SEQ_LEN - V
                    if typ == "cnt":
                        if eng == "dve":
                            n0 = raw0
                            n1 = raw1 - T2 * (t1 < 0)
                        else:
                            n0 = (raw0 + SEQ_LEN) / 2
                            # Sign: zeros give sign(0 - t') = -sign(t')
                            tp1 = np.nextafter(
                                t1.astype(np.float32), np.float32(np.inf)
                            ).astype(np.float64)
                            n1 = (raw1 + T2 * np.sign(tp1) + V) / 2
                        tgt[0, :, :, s] = n0.T.reshape(42, N_CORES)
                        tgt[1, 0:22, :, s] = n1.T.reshape(22, N_CORES)
                    else:
                        if eng == "dve":
                            r0 = raw0 - SEQ_LEN * t0
                            r1 = (raw1 - T2 * np.maximum(t1, 0) - V * t1)
                        else:
                            r0 = raw0
                            r1 = raw1 - T2 * np.maximum(-t1, 0)
                        tgt[0, :, :, s] = r0.T.reshape(42, N_CORES)
                        tgt[1, 0:22, :, s] = r1.T.reshape(22, N_CORES)
            else:
                for s in range(4):
                    v = acc[:, s * 32: s * 32 + 20]  # [ncores, 20]
                    t1 = biases[h1[22:42], self.off[i] + f][None, :]
                    if typ == "cnt":
                        n1 = v if eng == "dve" else (v + V) / 2
                        tgt[1, 22:42, :, s] = n1.T.reshape(20, N_CORES)
                    else:
                        r1 = v - V * t1 if eng == "dve" else v
                        tgt[1, 22:42, :, s] = r1.T.reshape(20, N_CORES)
        # assemble features
        for i, (d, p, nf) in enumerate(zip(DILS, PADS, NFPD)):
            V = SEQ_LEN - 2 * p
            h0, h1 = _halves(i)
            base = blk_off[i]
            s_arr = S[i]  # [2, 42, B] in b-global order? stored [2,42,B]
            for f in range(nf):
                t0 = biases[h0, self.off[i] + f]
                t1 = biases[h1, self.off[i] + f]
                n = N[(i, f)]
                r = R[(i, f)]
                for core in range(N_CORES):
                    for bb in range(B_CORE):
                        gb = core * B_CORE + bb
                        n0 = n[0, :, core, bb]
                        n1 = n[1, :, core, bb]
                        r0 = r[0, :, core, bb]
                        r1 = r[1, :, core, bb]
                        s0 = s_arr[0, :, gb]
                        s1 = s_arr[1, :, gb]
                        a0 = n0 / SEQ_LEN
                        a1 = n1 / V
                        d0 = s0 - SEQ_LEN * t0
                        d1 = s1 - V * t1
                        b0 = r0 / np.maximum(2 * r0 - d0, 1e-8)
                        b1 = r1 / np.maximum(2 * r1 - d1, 1e-8)
                        idx = np.arange(42)
                        out[gb, base + f + nf * idx] = a0
                        out[gb, base + 42 * nf + f + nf * idx] = b0
                        out[gb, base + 84 * nf + f + nf * idx] = a1
                        out[gb, base + 126 * nf + f + nf * idx] = b1
        return out.astype(np.float32)


PLAN = Plan2()
_NC_CACHE = {}


def _build_nc():
    import concourse.bacc as bacc
    import concourse.tile as tile
    from concourse import mybir
    import concourse.bass as bass

    nc = bacc.Bacc()
    x_pad = nc.dram_tensor("x_pad", [C_IN, B_CORE, LP2], mybir.dt.bfloat16,
                           kind="ExternalInput")
    x_pad_lo = nc.dram_tensor("x_pad_lo", [C_IN, B_CORE, LP2],
                              mybir.dt.bfloat16, kind="ExternalInput")
    w2 = nc.dram_tensor("w2", [ND, KDIM, 84], mybir.dt.bfloat16,
                        kind="ExternalInput")
    thr_in = nc.dram_tensor("thr_in", [128, PLAN.nthr], mybir.dt.float32,
                            kind="ExternalInput")
    acc_dve_out = nc.dram_tensor("acc_dve", [128, PLAN.ncol_dve],
                                 mybir.dt.float32, kind="ExternalOutput")
    acc_act_out = nc.dram_tensor("acc_act", [128, PLAN.ncol_act],
                                 mybir.dt.float32, kind="ExternalOutput")

    # group passes by (dil, tile)
    from collections import defaultdict
    tile_passes = defaultdict(list)
    for pa in PLAN.passes:
        tile_passes[(pa["dil"], pa["tile"])].append(pa)

    with tile.TileContext(nc) as tc:
        with tc.tile_pool(name="sb", bufs=1) as sb, \
             tc.tile_pool(name="slab", bufs=2) as slab_pool, \
             tc.tile_pool(name="ps", bufs=2, space="PSUM") as ps:
            w2_sb = sb.tile([KDIM, ND, 84], mybir.dt.bfloat16, tag="w2")
            z22 = sb.tile([KDIM, 22], mybir.dt.bfloat16, tag="z22")
            thr = sb.tile([128, PLAN.nthr], mybir.dt.float32, tag="thr")
            acc_dve = sb.tile([128, PLAN.ncol_dve], mybir.dt.float32, tag="accd")
            acc_act = sb.tile([128, PLAN.ncol_act], mybir.dt.float32, tag="acca")
            junk_d = sb.tile([128, SEQ_LEN], mybir.dt.bfloat16, tag="junkd")
            junk_a = sb.tile([128, SEQ_LEN], mybir.dt.bfloat16, tag="junka")

            nc.sync.dma_start(out=w2_sb, in_=w2[:, :, :].transpose([1, 0, 2]))
            nc.sync.dma_start(out=thr, in_=thr_in[:, :])
            nc.vector.memset(z22, 0.0)

            for i in DIL_ORDER:
                d, p, nf = DILS[i], PADS[i], NFPD[i]
                V = SEQ_LEN - 2 * p
                is32 = i in FP32_DILS

                def _gather(dst_tile, src_dram):
                    full = src_dram[:, :, :]
                    for c in range(C_IN):
                        src_ap = bass.AP(
                            tensor=full.tensor,
                            offset=PAD - 4 * d + c * B_CORE * LP2,
                            ap=[[d, NTAP], [LP2, B_CORE], [1, SEQ_LEN]],
                        )
                        dst_ap = bass.AP(
                            tensor=dst_tile.tensor,
                            offset=dst_tile.offset + c * XCOLS,
                            ap=[[C_IN * XCOLS, NTAP], [SEQ_LEN, B_CORE],
                                [1, SEQ_LEN]],
                        )
                        nc.sync.dma_start(out=dst_ap, in_=src_ap)

                xm = slab_pool.tile([KDIM, XCOLS], mybir.dt.bfloat16, tag="xm")
                _gather(xm, x_pad)
                xm_lo = None
                if is32:
                    xm_lo = slab_pool.tile([KDIM, XCOLS], mybir.dt.bfloat16,
                                           tag="xmlo")
                    _gather(xm_lo, x_pad_lo)

                lhs1 = w2_sb[:, i, 0:64]
                lhs2 = w2_sb[:, i, 64:84]

                tiles = {}
                for tname, samples in (("A", (0, 1)), ("B", (2, 3))):
                    pt = ps.tile([128, SEQ_LEN], mybir.dt.float32, tag="pt")
                    tiles[tname] = pt
                    for si, s in enumerate(samples):
                        x0 = s * SEQ_LEN
                        pb = si * 64
                        for k in range(4):
                            o = pt[pb:pb + 64, 512 * k:512 * (k + 1)]
                            nc.tensor.matmul(
                                o, lhs1,
                                xm[:, x0 + 512 * k: x0 + 512 * (k + 1)],
                                start=True, stop=not is32,
                                tile_position=(0, pb))
                            if is32:
                                nc.tensor.matmul(
                                    o, lhs1,
                                    xm_lo[:, x0 + 512 * k: x0 + 512 * (k + 1)],
                                    start=False, stop=True,
                                    tile_position=(0, pb))
                    # zero the h1-row tails [V, 2048) via zero-weight MMs
                    for pb in (0, 64):
                        zs = V
                        while zs < SEQ_LEN:
                            zl = min(512 * (zs // 512 + 1), SEQ_LEN) - zs
                            nc.tensor.matmul(
                                pt[pb:pb + 22, zs:zs + zl], z22,
                                xm[:, 0:zl], start=True, stop=True,
                                tile_position=(0, pb))
                            zs += zl
                    for pa in tile_passes[(i, tname)]:
                        _emit_pass(nc, mybir, pa, pt, SEQ_LEN, thr,
                                   acc_dve, acc_act, junk_d, junk_a)
                # tile C: rows 64:84 of each sample, width V
                pt = ps.tile([128, SEQ_LEN], mybir.dt.float32, tag="pt")
                for s in range(4):
                    x0 = s * SEQ_LEN
                    pb = s * 32
                    kv, rem = divmod(V, 512)
                    chunks = [(512 * k, 512) for k in range(kv)]
                    if rem:
                        chunks.append((512 * kv, rem))
                    for co, cw in chunks:
                        o = pt[pb:pb + 20, co:co + cw]
                        nc.tensor.matmul(
                            o, lhs2, xm[:, x0 + co: x0 + co + cw],
                            start=True, stop=not is32,
                            tile_position=(0, pb))
                        if is32:
                            nc.tensor.matmul(
                                o, lhs2, xm_lo[:, x0 + co: x0 + co + cw],
                                start=False, stop=True,
                                tile_position=(0, pb))
                for pa in tile_passes[(i, "C")]:
                    _emit_pass(nc, mybir, pa, pt, V, thr,
                               acc_dve, acc_act, junk_d, junk_a)

            nc.sync.dma_start(out=acc_dve_out[:, :], in_=acc_dve)
            nc.sync.dma_start(out=acc_act_out[:, :], in_=acc_act)
    nc.compile()
    return nc


def _emit_pass(nc, mybir, pa, pt, w, thr, acc_dve, acc_act, junk_d, junk_a):
    tcol = thr[:, pa["thr"]:pa["thr"] + 1]
    if pa["eng"] == "dve":
        acol = acc_dve[:, pa["acc"]:pa["acc"] + 1]
        op0 = (mybir.AluOpType.is_gt if pa["typ"] == "cnt"
               else mybir.AluOpType.max)
        nc.vector.tensor_scalar(out=junk_d[:, 0:w], in0=pt[:, 0:w],
                                scalar1=tcol, scalar2=None, op0=op0,
                                op1=mybir.AluOpType.add, accum_out=acol)
    else:
        acol = acc_act[:, pa["acc"]:pa["acc"] + 1]
        func = (mybir.ActivationFunctionType.Sign if pa["typ"] == "cnt"
                else mybir.ActivationFunctionType.Relu)
        nc.scalar.activation(out=junk_a[:, 0:w], in_=pt[:, 0:w], func=func,
                             bias=tcol, scale=1.0, accum_out=acol)


def _host_prep(x, kernels, channel_combinations):
    import ml_dtypes
    x = np.asarray(x, np.float32)
    kernels = np.asarray(kernels, np.float32)
    cc = np.asarray(channel_combinations, np.float32)
    x_pad = np.zeros((B, C_IN, LP2), np.float32)
    x_pad[:, :, PAD:PAD + SEQ_LEN] = x
    kern = kernels.reshape(C_IN, NUM_KERNELS, KERNEL_SIZE)  # [c, j, t]
    # w2t[i, row, col]: rows (t*9+c); cols 0..41 h0[j'] taps t0..8,
    # 42..83 h1[j'-42] at rows (t+4)*9+c
    w2t = np.zeros((ND, KDIM, 84), np.float32)
    for i in range(ND):
        h0, h1 = _halves(i)
        for t in range(KERNEL_SIZE):
            for c in range(C_IN):
                w2t[i, (t + 4) * 9 + c, 0:22] = cc[i, c, h1[0:22]] * kern[c, h1[0:22], t]
                w2t[i, t * 9 + c, 22:64] = cc[i, c, h0] * kern[c, h0, t]
                w2t[i, (t + 4) * 9 + c, 64:84] = cc[i, c, h1[22:42]] * kern[c, h1[22:42], t]
    return x_pad, w2t


def _make_in_maps(x_pad, w2t, thr, cores):
    import ml_dtypes
    w2bf = w2t.astype(ml_dtypes.bfloat16)
    in_maps = []
    for core in cores:
        xs = np.ascontiguousarray(
            x_pad[core * B_CORE:(core + 1) * B_CORE].transpose(1, 0, 2))
        xhi = xs.astype(ml_dtypes.bfloat16)
        xlo = (xs - xhi.astype(np.float32)).astype(ml_dtypes.bfloat16)
        in_maps.append({
            "x_pad": xhi,
            "x_pad_lo": xlo,
            "w2": w2bf,
            "thr_in": thr,
        })
    return in_maps


def kernel(x, kernels, channel_combinations, biases, _run_cores=None):
    import ml_dtypes
    from concourse.bass_utils import run_bass_kernel_spmd

    x_pad, w2t = _host_prep(x, kernels, channel_combinations)
    thr = PLAN.build_thresholds(np.asarray(biases, np.float32))

    if "nc" not in _NC_CACHE:
        _NC_CACHE["nc"] = _build_nc()
    nc = _NC_CACHE["nc"]

    cores = list(range(N_CORES)) if _run_cores is None else _run_cores
    in_maps = _make_in_maps(x_pad, w2t, thr, cores)
    res = run_bass_kernel_spmd(nc, in_maps, core_ids=cores)
    acc_dve = np.stack([r["acc_dve"] for r in res.results])
    acc_act = np.stack([r["acc_act"] for r in res.results])
    if _run_cores is not None:
        reps = N_CORES // len(cores)
        acc_dve = np.concatenate([acc_dve] * reps)
        acc_act = np.concatenate([acc_act] * reps)
    # host S on bf16-cast values (fp32 dils use fp32 values)
    S = _host_S_all(x_pad, w2t)
    return PLAN.decode(acc_dve, acc_act, biases, S)


def _x_eff(x_pad):
    """hi and hi+lo bf16-effective values of x_pad."""
    import ml_dtypes
    xhi = x_pad.astype(ml_dtypes.bfloat16).astype(np.float32)
    xlo = (x_pad - xhi).astype(ml_dtypes.bfloat16).astype(np.float32)
    return xhi, xhi + xlo


def _host_S_all(x_pad, w2t):
    import ml_dtypes
    xhi, xhilo = _x_eff(x_pad)
    wb = w2t.astype(ml_dtypes.bfloat16).astype(np.float64)
    S = {}
    S_bf = PLAN.host_S(xhi.astype(np.float64), wb)
    S_32 = None
    for i in range(ND):
        if i in FP32_DILS:
            if S_32 is None:
                S_32 = PLAN.host_S(xhilo.astype(np.float64), wb)
            S[i] = S_32[i]
        else:
            S[i] = S_bf[i]
    return S


def sim_accums(x, kernels, channel_combinations, biases):
    """Numpy simulation of device accumulators (decode validation)."""
    import ml_dtypes
    x_pad, w2t = _host_prep(x, kernels, channel_combinations)
    thr = PLAN.build_thresholds(np.asarray(biases, np.float32))
    xhi, xhilo = _x_eff(x_pad)
    wb = w2t.astype(ml_dtypes.bfloat16).astype(np.float32)
    acc_dve = np.zeros((N_CORES, 128, PLAN.ncol_dve), np.float32)
    acc_act = np.zeros((N_CORES, 128, PLAN.ncol_act), np.float32)
    for core in range(N_CORES):
        Ctiles = {}
        for i, (d, p, nf) in enumerate(zip(DILS, PADS, NFPD)):
            V = SEQ_LEN - 2 * p
            is32 = i in FP32_DILS
            xs = (xhilo if is32 else xhi)[core * B_CORE:(core + 1) * B_CORE]
            w = wb[i]
            # slab per sample: [117, 2048]
            C84 = np.zeros((4, 84, SEQ_LEN), np.float32)
            for s in range(B_CORE):
                xsl = np.zeros((KDIM, SEQ_LEN), np.float32)
                for t in range(NTAP):
                    o = PAD - 4 * d + d * t
                    for c in range(C_IN):
                        xsl[t * 9 + c] = xs[s, c, o:o + SEQ_LEN]
                C84[s] = w.T @ xsl
            # tiles
            tA = np.zeros((128, SEQ_LEN), np.float32)
            tB = np.zeros((128, SEQ_LEN), np.float32)
            tC = np.zeros((128, SEQ_LEN), np.float32)
            for si, s in enumerate((0, 1)):
                tA[si * 64: si * 64 + 64] = C84[s, 0:64]
            for si, s in enumerate((2, 3)):
                tB[si * 64: si * 64 + 64] = C84[s, 0:64]
            for s in range(4):
                tC[s * 32: s * 32 + 20, 0:V] = C84[s, 64:84, 0:V]
            # memset h1 tails in A/B
            for tt in (tA, tB):
                tt[0:22, V:] = 0.0
                tt[64:86, V:] = 0.0
            Ctiles[i] = dict(A=tA, B=tB, C=tC)
        for pa in PLAN.passes:
            i = pa["dil"]
            V = SEQ_LEN - 2 * PADS[i]
            w = SEQ_LEN if pa["tile"] in ("A", "B") else V
            T = Ctiles[i][pa["tile"]][:, 0:w]
            tcol = thr[:, pa["thr"]][:, None]
            if pa["eng"] == "dve":
                if pa["typ"] == "cnt":
                    v = (T > tcol).sum(1, dtype=np.float64)
                else:
                    v = np.maximum(T, tcol).sum(1, dtype=np.float64)
                acc_dve[core, :, pa["acc"]] = v.astype(np.float32)
            else:
                if pa["typ"] == "cnt":
                    v = np.sign(T + tcol).sum(1, dtype=np.float64)
                else:
                    v = np.maximum(T + tcol, 0).sum(1, dtype=np.float64)
                acc_act[core, :, pa["acc"]] = v.astype(np.float32)
    return acc_dve, acc_act

